# revision 5
# baseline (speedup 1.0000x reference)
"""Trainium2 Bass kernel for nn_MetaLayer (gnn_message_passing).

Strategy (8 NeuronCores, SPMD, two launches):
  Phase 1: shard edges contiguously across cores. Each core gathers
    x[row], x[col], face[fi0], face[fi1] rows via indirect DMA from
    replicated bf16 tables, PE-transposes tiles, and accumulates the
    6-term edge MLP in PSUM.  ReLU -> edge_new shard (fp32).
  Host: assemble edge_new, cast bf16, replicate.
  Phase 2: shard nodes/faces/graphs across cores. Sorted-by-destination
    indirect gathers + one-hot matmuls compute transposed segment sums
    (sent/recv/fs/fr) per 128-destination block; these feed the node/
    face MLP matmuls directly as lhsT.  Per-graph pooling via ones-column
    matmuls; global MLP at the end.  Face mask folds into the ReLU scale.
"""

import math
import os

import numpy as np
import ml_dtypes

BF16 = ml_dtypes.bfloat16

# problem sizes (hardcoded per spec)
N_NODES, N_EDGES, N_FACES, N_GRAPHS, D = 131072, 262144, 131072, 256, 128
M_CORES = 8
NPG, EPG, FPG = N_NODES // N_GRAPHS, N_EDGES // N_GRAPHS, N_FACES // N_GRAPHS

_LAST_RESULTS = {}  # test.py peeks here for exec times


# --------------------------------------------------------------------------
# numpy fallback / host reference (used when input layout assumptions fail)
# --------------------------------------------------------------------------
def _np_impl(x, edge_attr, u, face, W_edge, b_edge, W_node, b_node, W_face,
             b_face, W_glob, b_glob, edge_index, face_index, node_batch,
             edge_batch, face_batch, face_mask, num_nodes, num_edges,
             num_faces):
    x = np.asarray(x, np.float32)
    edge_attr = np.asarray(edge_attr, np.float32)
    u = np.asarray(u, np.float32)
    face = np.asarray(face, np.float32)
    row, col = np.asarray(edge_index[0]), np.asarray(edge_index[1])
    f0, f1 = np.asarray(face_index[0]), np.asarray(face_index[1])
    nn_, ne, nf, ng = x.shape[0], edge_attr.shape[0], face.shape[0], u.shape[0]
    ge = np.repeat(u, np.asarray(num_edges), axis=0)[:ne]
    ef = np.concatenate([edge_attr, x[row], x[col], ge, face[f0], face[f1]], axis=1)
    edge_attr = np.maximum(ef @ W_edge + b_edge, 0.0)
    sent = np.zeros((nn_, D), np.float32)
    np.add.at(sent, row, edge_attr)
    recv = np.zeros((nn_, D), np.float32)
    np.add.at(recv, col, edge_attr)
    gn = np.repeat(u, np.asarray(num_nodes), axis=0)[:nn_]
    x = np.maximum(np.concatenate([x, sent, recv, gn], axis=1) @ W_node + b_node, 0.0)
    fs = np.zeros((nf, D), np.float32)
    np.add.at(fs, f0, edge_attr)
    fr = np.zeros((nf, D), np.float32)
    np.add.at(fr, f1, edge_attr)
    gf = np.repeat(u, np.asarray(num_faces), axis=0)[:nf]
    face = np.maximum(np.concatenate([face, fs, fr, gf], axis=1) @ W_face + b_face, 0.0)
    face = np.where(np.asarray(face_mask)[:, None], 0.0, face).astype(np.float32)
    na = np.zeros((ng, D), np.float32)
    np.add.at(na, np.asarray(node_batch), x)
    ea = np.zeros((ng, D), np.float32)
    np.add.at(ea, np.asarray(edge_batch), edge_attr)
    fa = np.zeros((ng, D), np.float32)
    np.add.at(fa, np.asarray(face_batch), face)
    u = np.maximum(np.concatenate([u, na, ea, fa], axis=1) @ W_glob + b_glob, 0.0)
    return (x, edge_attr, u, face)


# --------------------------------------------------------------------------
# Phase 1 builder: edge update
# --------------------------------------------------------------------------
def build_phase1(n_rows, f_rows, epc, tiles_per_graph, gpc, chunk=32):
    """Per-core kernel: edge_new = relu(sum_i gathered_i @ W_i + bias_g).

    Inputs (per core):
      x_tab   [n_rows, 128] bf16   (replicated)
      f_tab   [f_rows, 128] bf16   (replicated)
      eaT     [128, epc]    bf16   (host pre-transposed edge_attr shard)
      w5      [128, 5*128]  bf16   (W1(ea), W2(xr), W3(xc), W5(f0), W6(f1))
      biasE   [gpc, 128]    bf16   (u @ W4 + b_edge, local graphs)
      ones1   [1, 128]      bf16
      gidx    [128, 4, KT]  int32  (gather indices; rel 0=row,1=col,2=f0,3=f1)
    Output:
      edge_new [epc, 128] fp32
    """
    import concourse.bass as bass
    from concourse import bacc, mybir
    import concourse.tile as tile
    from concourse.masks import make_identity
    dt = mybir.dt

    KT = epc // 128
    assert KT % chunk == 0
    nc = bacc.Bacc("TRN2", target_bir_lowering=False, debug=False)

    x_tab = nc.dram_tensor("x_tab", [n_rows, 128], dt.bfloat16, kind="ExternalInput").ap()
    f_tab = nc.dram_tensor("f_tab", [f_rows, 128], dt.bfloat16, kind="ExternalInput").ap()
    eaT_d = nc.dram_tensor("eaT", [128, epc], dt.bfloat16, kind="ExternalInput").ap()
    w5_d = nc.dram_tensor("w5", [128, 5 * 128], dt.bfloat16, kind="ExternalInput").ap()
    biasE_d = nc.dram_tensor("biasE", [1, gpc * 128], dt.bfloat16, kind="ExternalInput").ap()
    ones_d = nc.dram_tensor("ones1", [1, 128], dt.bfloat16, kind="ExternalInput").ap()
    gidx_d = nc.dram_tensor("gidx", [128, 4, KT], dt.int32, kind="ExternalInput").ap()
    out_e = nc.dram_tensor("edge_new", [epc, 128], dt.float32, kind="ExternalOutput").ap()

    with tile.TileContext(nc) as tc:
        with tc.tile_pool(name="const", bufs=1) as cpool, \
             tc.tile_pool(name="gather", bufs=2) as gpool, \
             tc.tile_pool(name="xt", bufs=4) as xtpool, \
             tc.tile_pool(name="stage", bufs=2) as spool, \
             tc.tile_pool(name="ptr", bufs=4, space="PSUM") as ptr_pool, \
             tc.tile_pool(name="pacc", bufs=2, space="PSUM") as pacc_pool:

            eaT_sb = cpool.tile([128, epc], dt.bfloat16, name="eaT_sb")
            nc.sync.dma_start(eaT_sb[:], eaT_d)
            w5_sb = cpool.tile([128, 5 * 128], dt.bfloat16, name="w5_sb")
            nc.sync.dma_start(w5_sb[:], w5_d)
            biasE_sb = cpool.tile([1, gpc * 128], dt.bfloat16, name="biasE_sb")
            nc.sync.dma_start(biasE_sb[:], biasE_d)
            ones_sb = cpool.tile([1, 128], dt.bfloat16, name="ones_sb")
            nc.sync.dma_start(ones_sb[:], ones_d)
            gidx_sb = cpool.tile([128, 4, KT], dt.int32, name="gidx_sb")
            nc.sync.dma_start(gidx_sb[:], gidx_d)
            ident_sb = cpool.tile([128, 128], dt.bfloat16, name="ident_sb")
            make_identity(nc, ident_sb[:])

            for c in range(KT // chunk):
                gbufs = []
                for r in range(4):
                    gb = gpool.tile([128, chunk, 128], dt.bfloat16, tag=f"g{r}",
                                    name=f"gb{r}")
                    tab = x_tab if r < 2 else f_tab
                    nc.gpsimd.indirect_dma_start(
                        out=gb[:], out_offset=None, in_=tab,
                        in_offset=bass.IndirectOffsetOnAxis(
                            ap=gidx_sb[:, r, c * chunk:(c + 1) * chunk], axis=0))
                    gbufs.append(gb)
                stage = spool.tile([128, chunk, 128], dt.float32, tag="stage",
                                   name="stage")
                for t in range(chunk):
                    tt = c * chunk + t
                    g_graph = tt // tiles_per_graph
                    acc = pacc_pool.tile([128, 128], dt.float32, tag="acc",
                                         name="acc")
                    nc.tensor.matmul(acc[:], eaT_sb[:, tt * 128:(tt + 1) * 128],
                                     w5_sb[:, 0:128], start=True, stop=False)
                    for r in range(4):
                        tr = ptr_pool.tile([128, 128], dt.bfloat16, tag="tr",
                                           name="tr")
                        nc.tensor.transpose(tr[:], gbufs[r][:, t, :], ident_sb[:])
                        xt = xtpool.tile([128, 128], dt.bfloat16, tag="xt",
                                         name="xt")
                        nc.vector.tensor_copy(xt[:], tr[:])
                        nc.tensor.matmul(acc[:], xt[:],
                                         w5_sb[:, (1 + r) * 128:(2 + r) * 128],
                                         start=False, stop=False)
                    nc.tensor.matmul(acc[:], ones_sb[:],
                                     biasE_sb[:, g_graph * 128:(g_graph + 1) * 128],
                                     start=False, stop=True)
                    nc.scalar.activation(stage[:, t, :], acc[:],
                                         mybir.ActivationFunctionType.Relu)
                dview = out_e[c * chunk * 128:(c + 1) * chunk * 128, :] \
                    .rearrange("(t p) f -> p t f", p=128)
                nc.sync.dma_start(dview, stage[:])
    nc.compile()
    return nc


# --------------------------------------------------------------------------
# Phase 2 builder: node / face / global updates
# --------------------------------------------------------------------------
def build_phase2(e_rows, npc, fpc, gpc, blocks_per_graph_n, blocks_per_graph_f,
                 tpb, epc, gchunk=8, ochunk=16, ech=16):
    """Per-core kernel.

    Inputs:
      e_tab    [e_rows, 128] bf16  (replicated edge_new, bf16)
      xT       [128, npc]   bf16   (host-transposed x shard)
      faceT    [128, fpc]   bf16
      ea_rows  [epc, 128]   bf16   (this core's contiguous edge_new rows)
      wn       [128, 3*128] bf16   (Wn1(x), Wn2(sent), Wn3(recv))
      wf       [128, 3*128] bf16   (Wf1(face), Wf2(fs), Wf3(fr))
      wg       [128, 4*128] bf16   (Wg1(u), Wg2(na), Wg3(ea), Wg4(fa))
      biasN    [gpc, 128] bf16,  biasF [gpc, 128] bf16,  bglob [1,128] bf16
      uT       [128, gpc] bf16
      ones1    [1, 128] bf16
      onescol  [128, 1] fp32
      onescol16[128, 1] bf16
      iota     [128, 128] fp32
      sidx     [128, 4, NBT] int32  (segment gather idx: sent,recv,fs,fr)
      sdloc    [128, 4, NBT] fp32   (local dest in block, -1 for pads)
      fmask    [128, fpc//128] fp32 (0.0 where face masked else 1.0)
    Outputs: x_new [npc,128] f32, face_new [fpc,128] f32, u_new [gpc,128] f32
    """
    import concourse.bass as bass
    from concourse import bacc, mybir
    import concourse.tile as tile
    dt = mybir.dt

    NBLK_N, NBLK_F = npc // 128, fpc // 128
    NBT = (NBLK_N) * tpb  # gather tile-columns per relation (nodes; faces same)
    assert NBLK_N == NBLK_F
    ETILES = epc // 128
    etiles_per_graph = epc // gpc // 128

    nc = bacc.Bacc("TRN2", target_bir_lowering=False, debug=False)

    e_tab = nc.dram_tensor("e_tab", [e_rows, 128], dt.bfloat16, kind="ExternalInput").ap()
    xT_d = nc.dram_tensor("xT", [128, npc], dt.bfloat16, kind="ExternalInput").ap()
    faceT_d = nc.dram_tensor("faceT", [128, fpc], dt.bfloat16, kind="ExternalInput").ap()
    ear_d = nc.dram_tensor("ea_rows", [epc, 128], dt.bfloat16, kind="ExternalInput").ap()
    wn_d = nc.dram_tensor("wn", [128, 3 * 128], dt.bfloat16, kind="ExternalInput").ap()
    wf_d = nc.dram_tensor("wf", [128, 3 * 128], dt.bfloat16, kind="ExternalInput").ap()
    wg_d = nc.dram_tensor("wg", [128, 4 * 128], dt.bfloat16, kind="ExternalInput").ap()
    biasN_d = nc.dram_tensor("biasN", [1, gpc * 128], dt.bfloat16, kind="ExternalInput").ap()
    biasF_d = nc.dram_tensor("biasF", [1, gpc * 128], dt.bfloat16, kind="ExternalInput").ap()
    bglob_d = nc.dram_tensor("bglob", [1, 128], dt.bfloat16, kind="ExternalInput").ap()
    uT_d = nc.dram_tensor("uT", [128, gpc], dt.bfloat16, kind="ExternalInput").ap()
    ones_d = nc.dram_tensor("ones1", [1, 128], dt.bfloat16, kind="ExternalInput").ap()
    onescol_d = nc.dram_tensor("onescol", [128, 1], dt.float32, kind="ExternalInput").ap()
    onescol16_d = nc.dram_tensor("onescol16", [128, 1], dt.bfloat16, kind="ExternalInput").ap()
    iota_d = nc.dram_tensor("iota", [128, 128], dt.float32, kind="ExternalInput").ap()
    sidx_d = nc.dram_tensor("sidx", [128, 4, NBT], dt.int32, kind="ExternalInput").ap()
    sdloc_d = nc.dram_tensor("sdloc", [128, 4, NBT], dt.float32, kind="ExternalInput").ap()
    fmask_d = nc.dram_tensor("fmask", [128, NBLK_F], dt.float32, kind="ExternalInput").ap()
    xnew_d = nc.dram_tensor("x_new", [npc, 128], dt.float32, kind="ExternalOutput").ap()
    fnew_d = nc.dram_tensor("face_new", [fpc, 128], dt.float32, kind="ExternalOutput").ap()
    unew_d = nc.dram_tensor("u_new", [gpc, 128], dt.float32, kind="ExternalOutput").ap()

    RELU = mybir.ActivationFunctionType.Relu

    with tile.TileContext(nc) as tc:
        with tc.tile_pool(name="const", bufs=1) as cpool, \
             tc.tile_pool(name="gather", bufs=2) as gpool, \
             tc.tile_pool(name="small", bufs=4) as smpool, \
             tc.tile_pool(name="stage", bufs=2) as spool, \
             tc.tile_pool(name="pseg", bufs=2, space="PSUM") as pseg_pool, \
             tc.tile_pool(name="pacc", bufs=2, space="PSUM") as pacc_pool, \
             tc.tile_pool(name="ppool", bufs=1, space="PSUM") as ppool_pool:

            xT_sb = cpool.tile([128, npc], dt.bfloat16, name="xT_sb")
            nc.sync.dma_start(xT_sb[:], xT_d)
            faceT_sb = cpool.tile([128, fpc], dt.bfloat16, name="faceT_sb")
            nc.sync.dma_start(faceT_sb[:], faceT_d)
            wn_sb = cpool.tile([128, 3 * 128], dt.bfloat16, name="wn_sb")
            nc.sync.dma_start(wn_sb[:], wn_d)
            wf_sb = cpool.tile([128, 3 * 128], dt.bfloat16, name="wf_sb")
            nc.sync.dma_start(wf_sb[:], wf_d)
            wg_sb = cpool.tile([128, 4 * 128], dt.bfloat16, name="wg_sb")
            nc.sync.dma_start(wg_sb[:], wg_d)
            biasN_sb = cpool.tile([1, gpc * 128], dt.bfloat16, name="biasN_sb")
            nc.sync.dma_start(biasN_sb[:], biasN_d)
            biasF_sb = cpool.tile([1, gpc * 128], dt.bfloat16, name="biasF_sb")
            nc.sync.dma_start(biasF_sb[:], biasF_d)
            bglob_sb = cpool.tile([1, 128], dt.bfloat16, name="bglob_sb")
            nc.sync.dma_start(bglob_sb[:], bglob_d)
            uT_sb = cpool.tile([128, gpc], dt.bfloat16, name="uT_sb")
            nc.sync.dma_start(uT_sb[:], uT_d)
            ones_sb = cpool.tile([1, 128], dt.bfloat16, name="ones_sb")
            nc.sync.dma_start(ones_sb[:], ones_d)
            onescol_sb = cpool.tile([128, 1], dt.float32, name="onescol_sb")
            nc.sync.dma_start(onescol_sb[:], onescol_d)
            onescol16_sb = cpool.tile([128, 1], dt.bfloat16, name="onescol16_sb")
            nc.sync.dma_start(onescol16_sb[:], onescol16_d)
            iota_sb = cpool.tile([128, 128], dt.float32, name="iota_sb")
            nc.sync.dma_start(iota_sb[:], iota_d)
            sidx_sb = cpool.tile([128, 4, NBT], dt.int32, name="sidx_sb")
            nc.sync.dma_start(sidx_sb[:], sidx_d)
            sdloc_sb = cpool.tile([128, 4, NBT], dt.float32, name="sdloc_sb")
            nc.sync.dma_start(sdloc_sb[:], sdloc_d)
            fmask_sb = cpool.tile([128, NBLK_F], dt.float32, name="fmask_sb")
            nc.sync.dma_start(fmask_sb[:], fmask_d)

            # persistent per-graph pooled (transposed) sums:
            # cols [0:gpc]=naT, [gpc:2gpc]=eaT, [2gpc:3gpc]=faT
            poolT = ppool_pool.tile([128, 3 * gpc], dt.float32, name="poolT")

            def seg_sum_T(rel, blk, gbuf, gb_off):
                """Accumulate transposed segment sum for 128-dest block blk of
                relation rel into a PSUM tile; returns SBUF bf16 copy."""
                pt = pseg_pool.tile([128, 128], dt.float32, tag="pseg",
                                    name="pseg")
                for j in range(tpb):
                    col = blk * tpb + j
                    onehot = smpool.tile([128, 128], dt.bfloat16, tag="onehot",
                                         name="onehot")
                    nc.vector.tensor_scalar(
                        out=onehot[:], in0=iota_sb[:],
                        scalar1=sdloc_sb[:, rel, col:col + 1], scalar2=None,
                        op0=mybir.AluOpType.is_equal)
                    nc.tensor.matmul(pt[:], gbuf[:, gb_off + j, :], onehot[:],
                                     start=(j == 0), stop=(j == tpb - 1))
                sb = smpool.tile([128, 128], dt.bfloat16, tag=f"segT{rel % 2}",
                                 name="segT")
                nc.vector.tensor_copy(sb[:], pt[:])
                return sb

            # ---- A: node blocks ----
            assert NBLK_N % gchunk == 0 and NBLK_N % ochunk == 0
            for oc in range(NBLK_N // ochunk):
                ostage = spool.tile([128, ochunk, 128], dt.float32, tag="ostage",
                                    name="ostage")
                for oi in range(ochunk):
                    blk = oc * ochunk + oi
                    # gather chunk boundary
                    if blk % gchunk == 0:
                        gc = blk // gchunk
                        gbufs = {}
                        for rel in (0, 1):
                            gb = gpool.tile([128, gchunk * tpb, 128],
                                            dt.bfloat16, tag=f"sg{rel}",
                                            name=f"sg{rel}")
                            nc.gpsimd.indirect_dma_start(
                                out=gb[:], out_offset=None, in_=e_tab,
                                in_offset=bass.IndirectOffsetOnAxis(
                                    ap=sidx_sb[:, rel,
                                               gc * gchunk * tpb:(gc + 1) * gchunk * tpb],
                                    axis=0))
                            gbufs[rel] = gb
                        cur_gbufs_n = gbufs
                    gb_off = (blk % gchunk) * tpb
                    g = blk // blocks_per_graph_n
                    sentT = seg_sum_T(0, blk, cur_gbufs_n[0], gb_off)
                    recvT = seg_sum_T(1, blk, cur_gbufs_n[1], gb_off)
                    acc = pacc_pool.tile([128, 128], dt.float32, tag="acc",
                                         name="acc")
                    nc.tensor.matmul(acc[:], xT_sb[:, blk * 128:(blk + 1) * 128],
                                     wn_sb[:, 0:128], start=True, stop=False)
                    nc.tensor.matmul(acc[:], sentT[:], wn_sb[:, 128:256],
                                     start=False, stop=False)
                    nc.tensor.matmul(acc[:], recvT[:], wn_sb[:, 256:384],
                                     start=False, stop=False)
                    nc.tensor.matmul(acc[:], ones_sb[:],
                                     biasN_sb[:, g * 128:(g + 1) * 128],
                                     start=False, stop=True)
                    nc.scalar.activation(ostage[:, oi, :], acc[:], RELU)
                    # pooled na^T contribution (fp32 matmul)
                    nc.tensor.matmul(poolT[:, g:g + 1], ostage[:, oi, :],
                                     onescol_sb[:],
                                     start=(blk % blocks_per_graph_n == 0),
                                     stop=(blk % blocks_per_graph_n ==
                                           blocks_per_graph_n - 1))
                dview = xnew_d[oc * ochunk * 128:(oc + 1) * ochunk * 128, :] \
                    .rearrange("(t p) f -> p t f", p=128)
                nc.sync.dma_start(dview, ostage[:])

            # ---- B: face blocks ----
            for oc in range(NBLK_F // ochunk):
                fstage = spool.tile([128, ochunk, 128], dt.float32, tag="fstage",
                                    name="fstage")
                for oi in range(ochunk):
                    blk = oc * ochunk + oi
                    if blk % gchunk == 0:
                        gc = blk // gchunk
                        gbufs = {}
                        for rel in (2, 3):
                            gb = gpool.tile([128, gchunk * tpb, 128],
                                            dt.bfloat16, tag=f"sg{rel}",
                                            name=f"sg{rel}")
                            nc.gpsimd.indirect_dma_start(
                                out=gb[:], out_offset=None, in_=e_tab,
                                in_offset=bass.IndirectOffsetOnAxis(
                                    ap=sidx_sb[:, rel,
                                               gc * gchunk * tpb:(gc + 1) * gchunk * tpb],
                                    axis=0))
                            gbufs[rel] = gb
                        cur_gbufs_f = gbufs
                    gb_off = (blk % gchunk) * tpb
                    g = blk // blocks_per_graph_f
                    fsT = seg_sum_T(2, blk, cur_gbufs_f[2], gb_off)
                    frT = seg_sum_T(3, blk, cur_gbufs_f[3], gb_off)
                    acc = pacc_pool.tile([128, 128], dt.float32, tag="acc",
                                         name="acc")
                    nc.tensor.matmul(acc[:], faceT_sb[:, blk * 128:(blk + 1) * 128],
                                     wf_sb[:, 0:128], start=True, stop=False)
                    nc.tensor.matmul(acc[:], fsT[:], wf_sb[:, 128:256],
                                     start=False, stop=False)
                    nc.tensor.matmul(acc[:], frT[:], wf_sb[:, 256:384],
                                     start=False, stop=False)
                    nc.tensor.matmul(acc[:], ones_sb[:],
                                     biasF_sb[:, g * 128:(g + 1) * 128],
                                     start=False, stop=True)
                    # relu(mask * (pre+bias)) == mask * relu(pre+bias)
                    nc.scalar.activation(fstage[:, oi, :], acc[:], RELU,
                                         scale=fmask_sb[:, blk:blk + 1])
                    nc.tensor.matmul(poolT[:, 2 * gpc + g:2 * gpc + g + 1],
                                     fstage[:, oi, :], onescol_sb[:],
                                     start=(blk % blocks_per_graph_f == 0),
                                     stop=(blk % blocks_per_graph_f ==
                                           blocks_per_graph_f - 1))
                dview = fnew_d[oc * ochunk * 128:(oc + 1) * ochunk * 128, :] \
                    .rearrange("(t p) f -> p t f", p=128)
                nc.sync.dma_start(dview, fstage[:])

            # ---- C: ea pooling over this core's contiguous edge rows ----
            ECH = min(ech, ETILES)
            assert ETILES % ECH == 0
            for c in range(ETILES // ECH):
                ebuf = gpool.tile([128, ECH, 128], dt.bfloat16, tag="ebuf",
                                  name="ebuf")
                dview = ear_d[c * ECH * 128:(c + 1) * ECH * 128, :] \
                    .rearrange("(t p) f -> p t f", p=128)
                nc.sync.dma_start(ebuf[:], dview)
                for t in range(ECH):
                    tt = c * ECH + t
                    g = tt // etiles_per_graph
                    j = tt % etiles_per_graph
                    nc.tensor.matmul(poolT[:, gpc + g:gpc + g + 1],
                                     ebuf[:, t, :], onescol16_sb[:],
                                     start=(j == 0),
                                     stop=(j == etiles_per_graph - 1))

            # ---- D: global update ----
            poolT_sb = smpool.tile([128, 3 * gpc], dt.bfloat16, tag="poolT_sb",
                                   name="poolT_sb", bufs=1)
            nc.vector.tensor_copy(poolT_sb[:], poolT[:])
            uacc = pacc_pool.tile([gpc, 128], dt.float32, tag="uacc", name="uacc")
            nc.tensor.matmul(uacc[:], uT_sb[:], wg_sb[:, 0:128],
                             start=True, stop=False)
            nc.tensor.matmul(uacc[:], poolT_sb[:, 0:gpc], wg_sb[:, 128:256],
                             start=False, stop=False)
            nc.tensor.matmul(uacc[:], poolT_sb[:, gpc:2 * gpc],
                             wg_sb[:, 256:384], start=False, stop=False)
            nc.tensor.matmul(uacc[:], poolT_sb[:, 2 * gpc:3 * gpc],
                             wg_sb[:, 384:512], start=False, stop=False)
            nc.tensor.matmul(uacc[:], ones_sb[:, 0:gpc], bglob_sb[:],
                             start=False, stop=True)
            unew_sb = smpool.tile([gpc, 128], dt.float32, tag="unew_sb",
                                  name="unew_sb", bufs=1)
            nc.scalar.activation(unew_sb[:], uacc[:], RELU)
            nc.sync.dma_start(unew_d, unew_sb[:])
    nc.compile()
    return nc


# --------------------------------------------------------------------------
# host-side index preparation
# --------------------------------------------------------------------------
def _gather_layout(idx_flat, ncols):
    """[ncols*128] -> [128, ncols] partition-major gather layout."""
    return np.ascontiguousarray(
        idx_flat.reshape(ncols, 128).T).astype(np.int32)


def _seg_lists(dest, lo, hi, tpb):
    """Edges e (global ids given implicitly by position in `dest`'s index
    array) whose dest lies in [lo, hi) sorted by dest, padded per
    128-dest block to tpb*128 slots.

    Returns (gidx [nblk*tpb*128] int64 edge ids, dloc same shape fp32)."""
    nblk = (hi - lo) // 128
    sel = np.nonzero((dest >= lo) & (dest < hi))[0]
    d = dest[sel] - lo
    order = np.argsort(d, kind="stable")
    sel, d = sel[order], d[order]
    blk = d >> 7
    counts = np.bincount(blk, minlength=nblk)
    cap = tpb * 128
    assert counts.max(initial=0) <= cap
    gidx = np.zeros(nblk * cap, np.int64)
    dloc = np.full(nblk * cap, -1.0, np.float32)
    starts = np.concatenate([[0], np.cumsum(counts)])[:-1]
    pos = (np.arange(len(d)) - starts[blk]) + blk * cap
    gidx[pos] = sel
    dloc[pos] = (d & 127).astype(np.float32)
    return gidx, dloc


# --------------------------------------------------------------------------
# main kernel
# --------------------------------------------------------------------------
def kernel(**inputs):
    x = np.asarray(inputs["x"], np.float32)
    edge_attr = np.asarray(inputs["edge_attr"], np.float32)
    u = np.asarray(inputs["u"], np.float32)
    face = np.asarray(inputs["face"], np.float32)
    W_edge = np.asarray(inputs["W_edge"], np.float32)
    b_edge = np.asarray(inputs["b_edge"], np.float32)
    W_node = np.asarray(inputs["W_node"], np.float32)
    b_node = np.asarray(inputs["b_node"], np.float32)
    W_face = np.asarray(inputs["W_face"], np.float32)
    b_face = np.asarray(inputs["b_face"], np.float32)
    W_glob = np.asarray(inputs["W_glob"], np.float32)
    b_glob = np.asarray(inputs["b_glob"], np.float32)
    edge_index = np.asarray(inputs["edge_index"]).astype(np.int64)
    face_index = np.asarray(inputs["face_index"]).astype(np.int64)
    node_batch = np.asarray(inputs["node_batch"]).astype(np.int64)
    edge_batch = np.asarray(inputs["edge_batch"]).astype(np.int64)
    face_batch = np.asarray(inputs["face_batch"]).astype(np.int64)
    face_mask = np.asarray(inputs["face_mask"]).astype(bool)
    num_nodes = np.asarray(inputs["num_nodes"]).astype(np.int64)
    num_edges = np.asarray(inputs["num_edges"]).astype(np.int64)
    num_faces = np.asarray(inputs["num_faces"]).astype(np.int64)

    nn_, ne, nf, ng = x.shape[0], edge_attr.shape[0], face.shape[0], u.shape[0]
    uniform = (
        nn_ == N_NODES and ne == N_EDGES and nf == N_FACES and ng == N_GRAPHS
        and x.shape[1] == D
        and np.all(num_nodes == NPG) and np.all(num_edges == EPG)
        and np.all(num_faces == FPG)
        and np.array_equal(node_batch, np.arange(nn_, dtype=np.int64) // NPG)
        and np.array_equal(edge_batch, np.arange(ne, dtype=np.int64) // EPG)
        and np.array_equal(face_batch, np.arange(nf, dtype=np.int64) // FPG)
    )
    if not uniform or os.environ.get("KERNEL_FORCE_NP"):
        return _np_impl(x, edge_attr, u, face, W_edge, b_edge, W_node, b_node,
                        W_face, b_face, W_glob, b_glob, edge_index, face_index,
                        node_batch, edge_batch, face_batch, face_mask,
                        num_nodes, num_edges, num_faces)

    from concourse import bass_utils

    EPC, NPC, FPC, GPC = ne // M_CORES, nn_ // M_CORES, nf // M_CORES, ng // M_CORES
    row, col = edge_index[0], edge_index[1]
    f0, f1 = face_index[0], face_index[1]

    # ---------------- phase 1 ----------------
    x16 = x.astype(BF16)
    face16 = face.astype(BF16)
    W16 = W_edge.astype(BF16)
    w5 = np.concatenate([W16[0:D], W16[D:2 * D], W16[2 * D:3 * D],
                         W16[4 * D:5 * D], W16[5 * D:6 * D]], axis=1)
    w5 = np.ascontiguousarray(w5)  # [128, 5*128]
    biasE = (u @ W_edge[3 * D:4 * D] + b_edge).astype(BF16)  # [B, 128]
    ones1 = np.ones((1, 128), BF16)
    KT = EPC // 128

    nc1 = build_phase1(nn_, nf, EPC, EPG // 128, GPC, chunk=32)
    in_maps1 = []
    for k in range(M_CORES):
        sl = slice(k * EPC, (k + 1) * EPC)
        gidx = np.stack([
            _gather_layout(row[sl], KT),
            _gather_layout(col[sl], KT),
            _gather_layout(f0[sl], KT),
            _gather_layout(f1[sl], KT),
        ], axis=1)  # [128, 4, KT]
        in_maps1.append({
            "x_tab": x16, "f_tab": face16,
            "eaT": np.ascontiguousarray(edge_attr[sl].T.astype(BF16)),
            "w5": w5,
            "biasE": np.ascontiguousarray(biasE[k * GPC:(k + 1) * GPC].reshape(1, -1)),
            "ones1": ones1,
            "gidx": np.ascontiguousarray(gidx),
        })
    r1 = bass_utils.run_bass_kernel_spmd(
        nc1, in_maps1, core_ids=list(range(M_CORES)),
        trace=bool(os.environ.get("KERNEL_TRACE")))
    _LAST_RESULTS["phase1"] = r1
    edge_new = np.concatenate([r1.results[k]["edge_new"]
                               for k in range(M_CORES)], axis=0)

    # ---------------- phase 2 ----------------
    e16 = edge_new.astype(BF16)
    biasN = (u @ W_node[3 * D:4 * D] + b_node).astype(BF16)
    biasF = (u @ W_face[3 * D:4 * D] + b_face).astype(BF16)
    Wn16, Wf16, Wg16 = W_node.astype(BF16), W_face.astype(BF16), W_glob.astype(BF16)
    wn = np.ascontiguousarray(np.concatenate(
        [Wn16[0:D], Wn16[D:2 * D], Wn16[2 * D:3 * D]], axis=1))
    wf = np.ascontiguousarray(np.concatenate(
        [Wf16[0:D], Wf16[D:2 * D], Wf16[2 * D:3 * D]], axis=1))
    wg = np.ascontiguousarray(np.concatenate(
        [Wg16[0:D], Wg16[D:2 * D], Wg16[2 * D:3 * D], Wg16[3 * D:4 * D]],
        axis=1))
    bglob = b_glob.reshape(1, 128).astype(BF16)
    onescol = np.ones((128, 1), np.float32)
    onescol16 = np.ones((128, 1), BF16)
    iota = np.tile(np.arange(128, dtype=np.float32)[None, :], (128, 1))

    # segment lists; tpb uniform across cores/relations
    NBLK = NPC // 128
    lists = {}
    maxload = 0
    for k in range(M_CORES):
        for rel, dest in enumerate((row, col, f0, f1)):
            lo = k * NPC if rel < 2 else k * FPC
            hi = lo + (NPC if rel < 2 else FPC)
            d = dest[(dest >= lo) & (dest < hi)] - lo
            if len(d):
                maxload = max(maxload, np.bincount(d >> 7, minlength=NBLK).max())
    tpb = max(1, math.ceil(maxload / 128))

    nc2 = build_phase2(ne, NPC, FPC, GPC, NPG // 128, FPG // 128, tpb, EPC,
                       gchunk=8, ochunk=16)
    in_maps2 = []
    for k in range(M_CORES):
        sidx = np.empty((128, 4, NBLK * tpb), np.int32)
        sdloc = np.empty((128, 4, NBLK * tpb), np.float32)
        for rel, dest in enumerate((row, col, f0, f1)):
            lo = k * NPC if rel < 2 else k * FPC
            gi, dl = _seg_lists(dest, lo, lo + (NPC if rel < 2 else FPC), tpb)
            sidx[:, rel, :] = _gather_layout(gi, NBLK * tpb)
            sdloc[:, rel, :] = np.ascontiguousarray(
                dl.reshape(NBLK * tpb, 128).T)
        nsl = slice(k * NPC, (k + 1) * NPC)
        fsl = slice(k * FPC, (k + 1) * FPC)
        esl = slice(k * EPC, (k + 1) * EPC)
        gsl = slice(k * GPC, (k + 1) * GPC)
        fmask = (~face_mask[fsl]).astype(np.float32).reshape(-1, 128).T
        in_maps2.append({
            "e_tab": e16,
            "xT": np.ascontiguousarray(x[nsl].T.astype(BF16)),
            "faceT": np.ascontiguousarray(face[fsl].T.astype(BF16)),
            "ea_rows": np.ascontiguousarray(e16[esl]),
            "wn": wn, "wf": wf, "wg": wg,
            "biasN": np.ascontiguousarray(biasN[gsl].reshape(1, -1)),
            "biasF": np.ascontiguousarray(biasF[gsl].reshape(1, -1)),
            "bglob": bglob,
            "uT": np.ascontiguousarray(u[gsl].T.astype(BF16)),
            "ones1": ones1,
            "onescol": onescol, "onescol16": onescol16, "iota": iota,
            "sidx": sidx, "sdloc": sdloc,
            "fmask": np.ascontiguousarray(fmask),
        })
    r2 = bass_utils.run_bass_kernel_spmd(
        nc2, in_maps2, core_ids=list(range(M_CORES)),
        trace=bool(os.environ.get("KERNEL_TRACE")))
    _LAST_RESULTS["phase2"] = r2

    x_new = np.concatenate([r2.results[k]["x_new"] for k in range(M_CORES)], axis=0)
    face_new = np.concatenate([r2.results[k]["face_new"] for k in range(M_CORES)], axis=0)
    u_new = np.concatenate([r2.results[k]["u_new"] for k in range(M_CORES)], axis=0)
    return (x_new, edge_new, u_new, face_new)
